# revision 16
# baseline (speedup 1.0000x reference)
"""Trainium2 Bass kernel: dilated multi-head self-attention with relative
positional embedding (window 5, dilation 2, per-head shifts, head 7 reuses
head 6 keys).

Sharding: 8 cores = 2 batches x 4 token-quarters (1024 tokens each, all 8
heads per core).  The host pre-transposes x and W (pure layout prep) so the
device spends its time on: QKV projections (fp32r matmuls at full PE rate),
banded QK scores per 112-token tile, band extraction via diagonal-stride DMA
reads, a 5-wide band softmax, a banded attention matrix rebuilt via DMA
scatter through pre-zeroed DRAM blocks, and AV matmuls.
"""
import sys

sys.path.insert(0, "/opt/trn_rl_repo")

import numpy as np
import concourse.bass as bass
from concourse import bacc
import concourse.mybir as mybir
import concourse.tile as tile
from concourse import bass_utils
from concourse.tile_rust import add_dep_helper
from contextlib import ExitStack

# ---------------- problem constants (hardcoded) ----------------
B, T, D = 2, 4096, 1024
H, HD, L = 8, 128, 5
LAYER = 1
DIL = 2 ** LAYER          # 2
PADR = DIL * (L // 2)     # 4  (reference's pad)
TP = T + 2 * PADR         # 4104
TAU = float(np.sqrt(HD))
SHIFTS = [0, 0, 0, 0, -2, -1, 1, 2]
SRC_K = [0, 1, 2, 3, 4, 5, 6, 6]
C0 = [s * DIL - PADR for s in SHIFTS]   # [-4,-4,-4,-4,-8,-6,-2,0]
NCORES = 8
TC = T // 4               # 1024 tokens per core
TS = 112                  # q-tile size
NTILE = 10
SKS = [min(k * TS, TC - TS) for k in range(NTILE)]  # 0..896, 912 (overlap)
HALO = 8
TKV = TC + 2 * HALO       # 1040
KH = 7                    # distinct k heads (head 7 reuses 6)
CLASS_OF = [0, 0, 0, 0, 1, 2, 3, 4]
ABUFS = [4, 2, 2, 2, 2]   # DRAM A-block rotation depth per class
ABASE = [0, 4, 6, 8, 10]
NABLK = 12
DCH = 8                   # D chunks of 128

F32 = mybir.dt.float32
F32R = mybir.dt.float32r

USE_F32R = True

_CACHE = {}


def _build_nc(debug=False):
    nc = bacc.Bacc(target_bir_lowering=False)
    fr = F32R if USE_F32R else F32

    # ---------- DRAM I/O ----------
    xTq_d = nc.dram_tensor("xTq", [DCH, 128, TC], fr, kind="ExternalInput")
    xTk_d = nc.dram_tensor("xTk", [DCH, 128, TKV], fr, kind="ExternalInput")
    xTv_d = nc.dram_tensor("xTv", [DCH, 128, TKV], fr, kind="ExternalInput")
    WTq_d = nc.dram_tensor("WTq", [DCH, 128, D], fr, kind="ExternalInput")
    WTk_d = nc.dram_tensor("WTk", [DCH, 128, D], fr, kind="ExternalInput")
    WTv_d = nc.dram_tensor("WTv", [DCH, 128, D], fr, kind="ExternalInput")
    G_d = nc.dram_tensor("G", [DCH, 128, H * L], fr, kind="ExternalInput")
    MASK_d = nc.dram_tensor("maskadd", [NTILE, TS, H * L], F32, kind="ExternalInput")
    BQ_d = nc.dram_tensor("bqs", [128, H], F32, kind="ExternalInput")
    BK_d = nc.dram_tensor("bk", [128, KH], F32, kind="ExternalInput")
    BV_d = nc.dram_tensor("bv", [128, D], F32, kind="ExternalInput")
    AZ_d = nc.dram_tensor("azero", [NABLK, 128, TS], F32, kind="ExternalInput")
    SS_d = nc.dram_tensor("sscratch", [6, TS, 128], F32, kind="ExternalInput")
    OUT_d = nc.dram_tensor("out", [TC, D], F32, kind="ExternalOutput")
    if debug:
        DQT_d = nc.dram_tensor("d_qT", [128, H, TC], F32, kind="ExternalOutput")
        DKT_d = nc.dram_tensor("d_kT", [128, KH, TKV], F32, kind="ExternalOutput")
        DQER_d = nc.dram_tensor("d_qer", [TS, NTILE, H * L], F32, kind="ExternalOutput")
        DVT_d = nc.dram_tensor("d_vt", [128, D], F32, kind="ExternalOutput")
        DSSB_d = nc.dram_tensor("d_ssb", [TS, 128], F32, kind="ExternalOutput")
        DBAND_d = nc.dram_tensor("d_band", [TS, 9], F32, kind="ExternalOutput")
        DEX_d = nc.dram_tensor("d_ex", [TS, L + 1], F32, kind="ExternalOutput")
        DASB_d = nc.dram_tensor("d_asb", [128, TS], F32, kind="ExternalOutput")
    ATTN_d = nc.dram_tensor("attn", [H, TC, L], F32, kind="ExternalOutput")

    with tile.TileContext(nc) as tc, ExitStack() as ctx:
        # ---------- persistent SBUF ----------
        xTv = ctx.enter_context(nc.sbuf_tensor("xTv_sb", [128, DCH, TKV], fr)).ap()
        qT = ctx.enter_context(nc.sbuf_tensor("qT_sb", [128, H, TC], F32)).ap()
        kT = ctx.enter_context(nc.sbuf_tensor("kT_sb", [128, KH, TKV], F32)).ap()
        G_sb = ctx.enter_context(nc.sbuf_tensor("G_sb", [128, DCH, H * L], fr)).ap()
        bqs = ctx.enter_context(nc.sbuf_tensor("bqs_sb", [128, H], F32)).ap()
        bks = ctx.enter_context(nc.sbuf_tensor("bks_sb", [128, KH], F32)).ap()
        bvb = ctx.enter_context(nc.sbuf_tensor("bvb_sb", [128, D], F32)).ap()
        qer_all = ctx.enter_context(nc.sbuf_tensor("qer_all_sb", [TS, NTILE, H * L], F32)).ap()
        scratch = ctx.enter_context(nc.psum_tensor("scratch_ps", [1, 2], F32)).ap()

        wpool = ctx.enter_context(tc.tile_pool(name="wt", bufs=2))
        bpool = ctx.enter_context(tc.tile_pool(name="small", bufs=4))

        proj_ps = ctx.enter_context(tc.tile_pool(name="proj_ps", bufs=2, space="PSUM"))
        qer_ps = ctx.enter_context(tc.tile_pool(name="qer_ps", bufs=1, space="PSUM"))
        att_ps = ctx.enter_context(tc.tile_pool(name="att_ps", bufs=4, space="PSUM"))

        flushsrc = ctx.enter_context(nc.sbuf_tensor("flushsrc", [1, 2], F32)).ap()
        nc.vector.memset(flushsrc, 0.0)

        def guarded_matmul(mm_fn, *fresh):
            # bacc's generate_event_semaphores legalizes multi-wait instrs
            return mm_fn()

        def pe_flush(reader_fn):
            f = nc.tensor.matmul(scratch[0:1, 0:1], flushsrc[0:1, 0:1],
                                 flushsrc[0:1, 0:1], start=True, stop=True,
                                 skip_group_check=True)
            r = reader_fn()
            add_dep_helper(r.ins, f.ins, sync=True, reason="pe-drain-flush")
            return r

        # ---------- load persistent inputs ----------
        def load3(dst, src_d, ncols):
            # dst sbuf [128, DCH, ncols] <- src dram [DCH, 128, ncols], per chunk
            for dc in range(DCH):
                nc.sync.dma_start(
                    out=dst[:, dc, :],
                    in_=bass.AP(tensor=src_d, offset=dc * 128 * ncols,
                                ap=[[ncols, 128], [1, ncols]]))

        load3(xTv, xTv_d, TKV)
        load3(G_sb, G_d, H * L)
        nc.sync.dma_start(out=bqs, in_=BQ_d.ap())
        nc.sync.dma_start(out=bks, in_=BK_d.ap())
        nc.sync.dma_start(out=bvb, in_=BV_d.ap())

        # ---------- phase 1: q/k projections + qer ----------
        def proj_head(dst3, w_dram, h, bias_col, xsrc, spans, first_extra):
            wt = wpool.tile([128, DCH, 128], fr, tag="wt")
            for dc in range(DCH):
                nc.sync.dma_start(
                    out=wt[:, dc, :],
                    in_=bass.AP(tensor=w_dram, offset=dc * 128 * D + h * 128,
                                ap=[[D, 128], [1, 128]]))
            fresh0 = [wt[:, 0, :]] + list(first_extra)
            for si, (soff, slen) in enumerate(spans):
                ps = proj_ps.tile([128, 512], F32, tag="proj")

                def group(ps=ps, soff=soff, slen=slen):
                    mm0 = None
                    for dc in range(DCH):
                        m = nc.tensor.matmul(ps[:, 0:slen], wt[:, dc, :],
                                             xsrc[:, dc, soff:soff + slen],
                                             start=(dc == 0), stop=(dc == DCH - 1))
                        if dc == 0:
                            mm0 = m
                    return mm0

                guarded_matmul(group, *(fresh0 if si == 0 else []))
                pe_flush(lambda ps=ps, soff=soff, slen=slen: nc.vector.tensor_scalar(
                    out=dst3[:, h, soff:soff + slen], in0=ps[:, 0:slen],
                    scalar1=bias_col, scalar2=None, op0=mybir.AluOpType.add))

        qspans = [(0, 512), (512, 512)]
        kspans = [(0, 512), (512, 512), (1024, 16)]
        with tc.tile_pool(name="xqk", bufs=1) as xqk_pool:
            xTq = xqk_pool.tile([128, DCH, TC], fr, tag="xq")
            xTk = xqk_pool.tile([128, DCH, TKV], fr, tag="xk")
            load3(xTq, xTq_d, TC)
            load3(xTk, xTk_d, TKV)
            for h in range(H):
                extra = [xTq[:, 0, :]] if h == 0 else []
                proj_head(qT, WTq_d, h, bqs[:, h:h + 1], xTq, qspans, extra)
            for h in range(KH):
                extra = [xTk[:, 0, :]] if h == 0 else []
                proj_head(kT, WTk_d, h, bks[:, h:h + 1], xTk, kspans, extra)

            # qer for all tiles (Er folded via host-precomputed G)
            for k in range(NTILE):
                sk = SKS[k]
                qe = qer_ps.tile([TS, H * L], F32, tag="qer")

                def qgroup(qe=qe, sk=sk):
                    mm0 = None
                    for dc in range(DCH):
                        m = nc.tensor.matmul(qe, xTq[:, dc, sk:sk + TS],
                                             G_sb[:, dc, :],
                                             start=(dc == 0), stop=(dc == DCH - 1))
                        if dc == 0:
                            mm0 = m
                    return mm0

                guarded_matmul(qgroup, *([G_sb[:, 0, :]] if k == 0 else []))
                msk = bpool.tile([TS, H * L], F32, tag="msk")
                nc.sync.dma_start(out=msk, in_=MASK_d.ap()[k])
                pe_flush(lambda k=k, qe=qe, msk=msk: nc.vector.tensor_tensor(
                    out=qer_all[:, k, :], in0=qe, in1=msk,
                    op=mybir.AluOpType.add))

        if debug:
            for h in range(H):
                nc.sync.dma_start(out=DQT_d.ap()[:, h, :], in_=qT[:, h, :])
            for h in range(KH):
                nc.sync.dma_start(out=DKT_d.ap()[:, h, :], in_=kT[:, h, :])
            for k in range(NTILE):
                nc.sync.dma_start(out=DQER_d.ap()[:, k, :], in_=qer_all[:, k, :])

        # phase-2 pools allocate into the space released by xqk_pool
        vpool = ctx.enter_context(tc.tile_pool(name="vsb", bufs=2))
        spool = ctx.enter_context(tc.tile_pool(name="ssb", bufs=3))
        apool = ctx.enter_context(tc.tile_pool(name="asb", bufs=4))
        stpool = ctx.enter_context(tc.tile_pool(name="stage", bufs=3))
        opool = ctx.enter_context(tc.tile_pool(name="osb", bufs=3))
        wv_pool = ctx.enter_context(tc.tile_pool(name="wvt", bufs=1))
        WvT = wv_pool.tile([128, DCH, D], fr, tag="wvt")
        load3(WvT, WTv_d, D)

        # ---------- phase 2: per-tile v projection + attention ----------
        acnt = [0] * 5
        for k in range(NTILE):
            sk = SKS[k]
            vt = vpool.tile([128, D], F32, tag="v")
            for si, soff in enumerate((0, 512)):
                ps = proj_ps.tile([128, 512], F32, tag="proj")

                def vgroup(ps=ps, sk=sk, soff=soff):
                    mm0 = None
                    for dc in range(DCH):
                        m = nc.tensor.matmul(ps, xTv[:, dc, sk:sk + 128],
                                             WvT[:, dc, soff:soff + 512],
                                             start=(dc == 0), stop=(dc == DCH - 1))
                        if dc == 0:
                            mm0 = m
                    return mm0

                extra = [WvT[:, 0, :]] if (k == 0 and si == 0) else []
                guarded_matmul(vgroup, *(([xTv[:, 0, :]] if k == 0 else []) + extra))
                pe_flush(lambda vt=vt, ps=ps, soff=soff: nc.vector.tensor_tensor(
                    out=vt[:, soff:soff + 512], in0=ps,
                    in1=bvb[:, soff:soff + 512],
                    op=mybir.AluOpType.add))

            stg = stpool.tile([TS, H * L], F32, tag="stg")
            for h in range(H):
                hs = SRC_K[h]
                cls = CLASS_OF[h]
                coff = C0[h] + 8
                sps = att_ps.tile([TS, 128], F32, tag="att")
                sfresh = [qT[:, h, sk:sk + 1], kT[:, hs, sk:sk + 1]] if k == 0 else []
                guarded_matmul(
                    lambda sps=sps, h=h, hs=hs, sk=sk: nc.tensor.matmul(
                        sps, qT[:, h, sk:sk + TS], kT[:, hs, sk:sk + 128],
                        start=True, stop=True),
                    *sfresh)
                ssb = spool.tile([TS, 128], F32, tag="ssb")
                pe_flush(lambda ssb=ssb, sps=sps: nc.scalar.copy(ssb, sps))
                sring = (k * H + h) % 6
                nc.scalar.dma_start(
                    out=bass.AP(tensor=SS_d, offset=sring * TS * 128,
                                ap=[[128, TS], [1, 128]]),
                    in_=ssb)
                band = bpool.tile([TS, 9], F32, tag="band")
                nc.scalar.dma_start(out=band, in_=bass.AP(
                    tensor=SS_d, offset=sring * TS * 128 + coff,
                    ap=[[129, TS], [1, 9]]))
                work5 = bpool.tile([TS, L], F32, tag="work5")
                nc.gpsimd.tensor_tensor(out=work5, in0=band[:, 0:9:2],
                                        in1=qer_all[:, k, h * L:h * L + L],
                                        op=mybir.AluOpType.add)
                ex = bpool.tile([TS, L + 1], F32, tag="ex")
                nc.scalar.activation(out=ex[:, 0:L], in_=work5,
                                     func=mybir.ActivationFunctionType.Exp,
                                     accum_out=ex[:, L:L + 1])
                rr = bpool.tile([TS, 1], F32, tag="rr")
                nc.vector.reciprocal(out=rr, in_=ex[:, L:L + 1])
                nc.scalar.mul(out=stg[:, h * L:h * L + L], in_=ex[:, 0:L],
                              mul=rr)
                # scatter attn5 -> pre-zeroed DRAM block (banded, transposed)
                blk = ABASE[cls] + (acnt[cls] % ABUFS[cls])
                acnt[cls] += 1
                blk_off = blk * 128 * TS
                nc.scalar.dma_start(
                    out=bass.AP(tensor=AZ_d, offset=blk_off + coff * TS,
                                ap=[[TS + 1, TS], [2 * TS, L], [1, 1]]),
                    in_=bass.AP(tensor=stg.tensor,
                                offset=stg.offset + h * L,
                                ap=[[H * L, TS], [1, L], [1, 1]]))
                asb = apool.tile([128, TS], F32, tag="asb")
                nc.scalar.dma_start(out=asb, in_=bass.AP(
                    tensor=AZ_d, offset=blk_off, ap=[[TS, 128], [1, TS]]))
                # AV: out[t, d] = sum_{t'} A[t', t] * v[t', d]
                ops = att_ps.tile([TS, 128], F32, tag="att")
                avfresh = [asb] + ([vt[:, h * 128:h * 128 + 1]] if h == 0 else [])
                guarded_matmul(
                    lambda ops=ops, h=h, asb=asb, vt=vt: nc.tensor.matmul(
                        ops, asb, vt[:, h * 128:(h + 1) * 128],
                        start=True, stop=True),
                    *avfresh)
                if debug and k == 3 and h == 0:
                    nc.sync.dma_start(out=DSSB_d.ap(), in_=ssb)
                    nc.sync.dma_start(out=DBAND_d.ap(), in_=band)
                    nc.sync.dma_start(out=DEX_d.ap(), in_=ex)
                    nc.sync.dma_start(out=DASB_d.ap(), in_=asb)
                    nc.sync.dma_start(out=DVT_d.ap(), in_=vt)
                osb = opool.tile([TS, 128], F32, tag="osb")
                pe_flush(lambda osb=osb, ops=ops: nc.scalar.copy(osb, ops))
                nc.scalar.dma_start(
                    out=OUT_d.ap()[sk:sk + TS, h * 128:(h + 1) * 128], in_=osb)
            nc.scalar.dma_start(
                out=bass.AP(tensor=ATTN_d, offset=sk * L,
                            ap=[[L, TS], [TC * L, H], [1, L]]),
                in_=bass.AP(tensor=stg.tensor, offset=stg.offset,
                            ap=[[H * L, TS], [L, H], [1, L]]))
    # fp32r matmuls are self-loading (no InstLdweights); bacc's
    # move_matmul_waits_to_ldweights would drop their excess waits entirely.
    # Skip it and let generate_event_semaphores split multi-waits instead.
    nc.move_matmul_waits_to_ldweights = lambda: None
    nc.compile()
    return nc


def _round_f32r(a):
    """fp32r operands must be pre-rounded; TRN2 fp32r keeps a tf32-like
    reduced mantissa.  Round-to-nearest-even at bit 13 by default; the
    exact width is patched by _set_f32r_bits() once measured."""
    b = np.ascontiguousarray(a, dtype=np.float32).view(np.uint32).copy()
    nbits = _CACHE.get("f32r_drop_bits", 13)
    if nbits == 0:
        return b.view(np.float32)
    half = np.uint32(1 << (nbits - 1))
    lsb = (b >> np.uint32(nbits)) & np.uint32(1)
    b = b + (half - np.uint32(1)) + lsb
    b &= np.uint32(0xFFFFFFFF) << np.uint32(nbits)
    return b.view(np.float32)


def _host_prep(query, key, value, Wq, bq, Wk, bk, Wv, bv, Er):
    tau = TAU
    Wqs = Wq / tau
    bqs_host = (bq / tau).astype(np.float32)
    G = np.zeros((D, H * L), np.float32)
    qer_bias = np.zeros((H, L), np.float32)
    for h in range(H):
        G[:, h * L:(h + 1) * L] = Wqs[h * HD:(h + 1) * HD, :].T @ Er[h]
        qer_bias[h] = bqs_host[h * HD:(h + 1) * HD] @ Er[h]

    def t8(a, ncols):   # [ncols, D] -> [DCH, 128, ncols]
        return np.ascontiguousarray(a.T).reshape(DCH, 128, ncols)

    rnd = lambda x: np.ascontiguousarray(x, np.float32)
    WTq_h = rnd(t8(Wqs, D))
    WTk_h = rnd(t8(Wk, D))
    WTv_h = rnd(t8(Wv, D))
    G_h = rnd(G.reshape(DCH, 128, H * L))
    az = np.zeros((NABLK, 128, TS), np.float32)
    ss_scratch = np.zeros((6, TS, 128), np.float32)

    in_maps = []
    for core in range(NCORES):
        b = core // 4
        t0c = TC * (core % 4)
        xq = query[b, t0c:t0c + TC, :]
        xk = np.zeros((TKV, D), np.float32)
        xv = np.zeros((TKV, D), np.float32)
        lo, hi = t0c - HALO, t0c + TC + HALO
        slo, shi = max(lo, 0), min(hi, T)
        xk[slo - lo:shi - lo] = key[b, slo:shi]
        xv[slo - lo:shi - lo] = value[b, slo:shi]

        mask = np.zeros((NTILE, TS, H * L), np.float32)
        for k in range(NTILE):
            g = t0c + SKS[k] + np.arange(TS)[:, None]
            for h in range(H):
                idx = (g + (SHIFTS[h] + np.arange(L)[None, :]) * DIL) % TP
                valid = (idx >= PADR) & (idx < T + PADR)
                mask[k, :, h * L:(h + 1) * L] = np.where(
                    valid, qer_bias[h][None, :], np.float32(-1e30))

        in_maps.append({
            "xTq": rnd(t8(xq, TC)),
            "xTk": rnd(t8(xk, TKV)),
            "xTv": rnd(t8(xv, TKV)),
            "WTq": WTq_h, "WTk": WTk_h, "WTv": WTv_h,
            "G": G_h,
            "maskadd": mask,
            "bqs": np.ascontiguousarray(bqs_host.reshape(H, 128).T),
            "bk": np.ascontiguousarray(np.asarray(bk, np.float32)[:KH * 128].reshape(KH, 128).T),
            "bv": np.ascontiguousarray(np.broadcast_to(np.asarray(bv, np.float32), (128, D))),
            "azero": az,
            "sscratch": ss_scratch,
        })
    return in_maps


def kernel(query, key, value, Wq, bq, Wk, bk, Wv, bv, Er, layer, **run_kwargs):
    assert int(layer) == LAYER
    in_maps = _host_prep(
        np.asarray(query, np.float32), np.asarray(key, np.float32),
        np.asarray(value, np.float32),
        np.asarray(Wq, np.float32), np.asarray(bq, np.float32),
        np.asarray(Wk, np.float32), np.asarray(bk, np.float32),
        np.asarray(Wv, np.float32), np.asarray(bv, np.float32),
        np.asarray(Er, np.float32))
    if "nc" not in _CACHE:
        _CACHE["nc"] = _build_nc()
    nc = _CACHE["nc"]
    res = bass_utils.run_bass_kernel_spmd(nc, in_maps, core_ids=list(range(NCORES)),
                                          **run_kwargs)
    out = np.zeros((B, T, D), np.float32)
    attn = np.zeros((B, H, T, 1, L), np.float32)
    for core in range(NCORES):
        b = core // 4
        t0c = TC * (core % 4)
        out[b, t0c:t0c + TC] = res.results[core]["out"]
        attn[b, :, t0c:t0c + TC, 0, :] = res.results[core]["attn"]
    kernel.last_result = res
    return out, attn


# revision 20
# speedup vs baseline: 2.8947x; 2.8947x over previous
"""Trainium2 Bass kernel: dilated multi-head self-attention with relative
positional embedding (window 5, dilation 2, per-head shifts, head 7 reuses
head 6 keys).

Sharding: 8 cores = 2 batches x 4 token-quarters (1024 tokens each, all 8
heads per core).  The host pre-transposes x and W (pure layout prep) so the
device spends its time on: QKV projections (fp32r matmuls at full PE rate),
banded QK scores per 112-token tile, band extraction via diagonal-stride DMA
reads, a 5-wide band softmax, a banded attention matrix rebuilt via DMA
scatter through pre-zeroed DRAM blocks, and AV matmuls.
"""
import sys

sys.path.insert(0, "/opt/trn_rl_repo")

import numpy as np
import concourse.bass as bass
from concourse import bacc
import concourse.mybir as mybir
import concourse.tile as tile
from concourse import bass_utils
from concourse.tile_rust import add_dep_helper
from contextlib import ExitStack

# ---------------- problem constants (hardcoded) ----------------
B, T, D = 2, 4096, 1024
H, HD, L = 8, 128, 5
LAYER = 1
DIL = 2 ** LAYER          # 2
PADR = DIL * (L // 2)     # 4  (reference's pad)
TP = T + 2 * PADR         # 4104
TAU = float(np.sqrt(HD))
SHIFTS = [0, 0, 0, 0, -2, -1, 1, 2]
SRC_K = [0, 1, 2, 3, 4, 5, 6, 6]
C0 = [s * DIL - PADR for s in SHIFTS]   # [-4,-4,-4,-4,-8,-6,-2,0]
NCORES = 8
TC = T // 4               # 1024 tokens per core
TS = 112                  # q-tile size
NTILE = 10
SKS = [min(k * TS, TC - TS) for k in range(NTILE)]  # 0..896, 912 (overlap)
HALO = 8
TKV = TC + 2 * HALO       # 1040
KH = 7                    # distinct k heads (head 7 reuses 6)
CLASS_OF = [0, 0, 0, 0, 1, 2, 3, 4]
ABUFS = [10, 4, 4, 4, 4]  # DRAM A-block rotation depth per class
ABASE = [0, 10, 14, 18, 22]
NABLK = 26
DCH = 8                   # D chunks of 128

F32 = mybir.dt.float32
F32R = mybir.dt.float32r

USE_F32R = True

_CACHE = {}


def _build_nc(debug=False):
    nc = bacc.Bacc(target_bir_lowering=False)
    fr = F32R if USE_F32R else F32

    # ---------- DRAM I/O ----------
    xTq_d = nc.dram_tensor("xTq", [DCH, 128, TC], fr, kind="ExternalInput")
    xTk_d = nc.dram_tensor("xTk", [DCH, 128, TKV], fr, kind="ExternalInput")
    xTv_d = nc.dram_tensor("xTv", [DCH, 128, TKV], fr, kind="ExternalInput")
    WTq_d = nc.dram_tensor("WTq", [DCH, 128, D], fr, kind="ExternalInput")
    WTk_d = nc.dram_tensor("WTk", [DCH, 128, D], fr, kind="ExternalInput")
    WTv_d = nc.dram_tensor("WTv", [DCH, 128, D], fr, kind="ExternalInput")
    G_d = nc.dram_tensor("G", [DCH, 128, H * L], fr, kind="ExternalInput")
    MASK_d = nc.dram_tensor("maskadd", [NTILE, TS, H * L], F32, kind="ExternalInput")
    BQ_d = nc.dram_tensor("bqs", [128, H], F32, kind="ExternalInput")
    BK_d = nc.dram_tensor("bk", [128, KH], F32, kind="ExternalInput")
    BV_d = nc.dram_tensor("bv", [128, D], F32, kind="ExternalInput")
    AZ_d = nc.dram_tensor("azero", [NABLK, 128, TS], F32, kind="ExternalInput")
    SS_d = nc.dram_tensor("sscratch", [16, TS, 1024], F32, kind="ExternalInput")
    OUT_d = nc.dram_tensor("out", [TC, D], F32, kind="ExternalOutput")
    if debug:
        DQT_d = nc.dram_tensor("d_qT", [128, H, TC], F32, kind="ExternalOutput")
        DKT_d = nc.dram_tensor("d_kT", [128, KH, TKV], F32, kind="ExternalOutput")
        DQER_d = nc.dram_tensor("d_qer", [TS, NTILE, H * L], F32, kind="ExternalOutput")
        DVT_d = nc.dram_tensor("d_vt", [128, D], F32, kind="ExternalOutput")
        DSSB_d = nc.dram_tensor("d_ssb", [TS, 128], F32, kind="ExternalOutput")
        DBAND_d = nc.dram_tensor("d_band", [TS, 9], F32, kind="ExternalOutput")
        DEX_d = nc.dram_tensor("d_ex", [TS, L + 1], F32, kind="ExternalOutput")
        DASB_d = nc.dram_tensor("d_asb", [128, TS], F32, kind="ExternalOutput")
    ATTN_d = nc.dram_tensor("attn", [H, TC, L], F32, kind="ExternalOutput")

    with tile.TileContext(nc) as tc, ExitStack() as ctx:
        # ---------- persistent SBUF ----------
        xTv = ctx.enter_context(nc.sbuf_tensor("xTv_sb", [128, DCH, TKV], fr)).ap()
        qT = ctx.enter_context(nc.sbuf_tensor("qT_sb", [128, H, TC], F32)).ap()
        kT = ctx.enter_context(nc.sbuf_tensor("kT_sb", [128, KH, TKV], F32)).ap()
        G_sb = ctx.enter_context(nc.sbuf_tensor("G_sb", [128, DCH, H * L], fr)).ap()
        bqs = ctx.enter_context(nc.sbuf_tensor("bqs_sb", [128, H], F32)).ap()
        bks = ctx.enter_context(nc.sbuf_tensor("bks_sb", [128, KH], F32)).ap()
        bvb = ctx.enter_context(nc.sbuf_tensor("bvb_sb", [128, D], F32)).ap()
        qer_all = ctx.enter_context(nc.sbuf_tensor("qer_all_sb", [TS, NTILE, H * L], F32)).ap()
        scratch = ctx.enter_context(nc.psum_tensor("scratch_ps", [1, 2], F32)).ap()

        wpool = ctx.enter_context(tc.tile_pool(name="wt", bufs=3))
        bpool = ctx.enter_context(tc.tile_pool(name="small", bufs=6))

        proj_ps = ctx.enter_context(tc.tile_pool(name="proj_ps", bufs=2, space="PSUM"))
        qer_ps = ctx.enter_context(tc.tile_pool(name="qer_ps", bufs=1, space="PSUM"))
        att_ps = ctx.enter_context(tc.tile_pool(name="att_ps", bufs=4, space="PSUM"))

        flushsrc = ctx.enter_context(nc.sbuf_tensor("flushsrc", [1, 2], F32)).ap()
        nc.vector.memset(flushsrc, 0.0)

        def guarded_matmul(mm_fn, *fresh):
            # bacc's generate_event_semaphores legalizes multi-wait instrs
            return mm_fn()

        def pe_flush(reader_fn):
            f = nc.tensor.matmul(scratch[0:1, 0:1], flushsrc[0:1, 0:1],
                                 flushsrc[0:1, 0:1], start=True, stop=True,
                                 skip_group_check=True)
            r = reader_fn()
            add_dep_helper(r.ins, f.ins, sync=True, reason="pe-drain-flush")
            return r

        # ---------- load persistent inputs ----------
        def load3(dst, src_d, ncols):
            # dst sbuf [128, DCH, ncols] <- src dram [DCH, 128, ncols], per chunk
            for dc in range(DCH):
                nc.sync.dma_start(
                    out=dst[:, dc, :],
                    in_=bass.AP(tensor=src_d, offset=dc * 128 * ncols,
                                ap=[[ncols, 128], [1, ncols]]))

        load3(xTv, xTv_d, TKV)
        load3(G_sb, G_d, H * L)
        nc.sync.dma_start(out=bqs, in_=BQ_d.ap())
        nc.sync.dma_start(out=bks, in_=BK_d.ap())
        nc.sync.dma_start(out=bvb, in_=BV_d.ap())

        # ---------- phase 1: q/k projections + qer ----------
        def proj_head(dst3, w_dram, h, bias_col, xsrc, spans, first_extra):
            wt = wpool.tile([128, DCH, 128], fr, tag="wt")
            for dc in range(DCH):
                nc.sync.dma_start(
                    out=wt[:, dc, :],
                    in_=bass.AP(tensor=w_dram, offset=dc * 128 * D + h * 128,
                                ap=[[D, 128], [1, 128]]))
            fresh0 = [wt[:, 0, :]] + list(first_extra)
            for si, (soff, slen) in enumerate(spans):
                ps = proj_ps.tile([128, 512], F32, tag="proj")

                def group(ps=ps, soff=soff, slen=slen):
                    mm0 = None
                    for dc in range(DCH):
                        m = nc.tensor.matmul(ps[:, 0:slen], wt[:, dc, :],
                                             xsrc[:, dc, soff:soff + slen],
                                             start=(dc == 0), stop=(dc == DCH - 1))
                        if dc == 0:
                            mm0 = m
                    return mm0

                guarded_matmul(group, *(fresh0 if si == 0 else []))
                pe_flush(lambda ps=ps, soff=soff, slen=slen: nc.vector.tensor_scalar(
                    out=dst3[:, h, soff:soff + slen], in0=ps[:, 0:slen],
                    scalar1=bias_col, scalar2=None, op0=mybir.AluOpType.add))

        qspans = [(0, 512), (512, 512)]
        kspans = [(0, 512), (512, 512), (1024, 16)]
        with tc.tile_pool(name="xqk", bufs=1) as xqk_pool:
            xTq = xqk_pool.tile([128, DCH, TC], fr, tag="xq")
            xTk = xqk_pool.tile([128, DCH, TKV], fr, tag="xk")
            load3(xTq, xTq_d, TC)
            load3(xTk, xTk_d, TKV)
            for h in range(H):
                extra = [xTq[:, 0, :]] if h == 0 else []
                proj_head(qT, WTq_d, h, bqs[:, h:h + 1], xTq, qspans, extra)
            for h in range(KH):
                extra = [xTk[:, 0, :]] if h == 0 else []
                proj_head(kT, WTk_d, h, bks[:, h:h + 1], xTk, kspans, extra)

            # qer for all tiles (Er folded via host-precomputed G)
            for k in range(NTILE):
                sk = SKS[k]
                qe = qer_ps.tile([TS, H * L], F32, tag="qer")

                def qgroup(qe=qe, sk=sk):
                    mm0 = None
                    for dc in range(DCH):
                        m = nc.tensor.matmul(qe, xTq[:, dc, sk:sk + TS],
                                             G_sb[:, dc, :],
                                             start=(dc == 0), stop=(dc == DCH - 1))
                        if dc == 0:
                            mm0 = m
                    return mm0

                guarded_matmul(qgroup, *([G_sb[:, 0, :]] if k == 0 else []))
                msk = bpool.tile([TS, H * L], F32, tag="msk")
                nc.sync.dma_start(out=msk, in_=MASK_d.ap()[k])
                pe_flush(lambda k=k, qe=qe, msk=msk: nc.vector.tensor_tensor(
                    out=qer_all[:, k, :], in0=qe, in1=msk,
                    op=mybir.AluOpType.add))

        if debug:
            for h in range(H):
                nc.sync.dma_start(out=DQT_d.ap()[:, h, :], in_=qT[:, h, :])
            for h in range(KH):
                nc.sync.dma_start(out=DKT_d.ap()[:, h, :], in_=kT[:, h, :])
            for k in range(NTILE):
                nc.sync.dma_start(out=DQER_d.ap()[:, k, :], in_=qer_all[:, k, :])

        # phase-2 pools allocate into the space released by xqk_pool
        vpool = ctx.enter_context(tc.tile_pool(name="vsb", bufs=3))
        spool = ctx.enter_context(tc.tile_pool(name="ssb", bufs=3))
        apool = ctx.enter_context(tc.tile_pool(name="asb", bufs=6))
        stpool = ctx.enter_context(tc.tile_pool(name="stage", bufs=4))
        opool = ctx.enter_context(tc.tile_pool(name="osb", bufs=3))
        wv_pool = ctx.enter_context(tc.tile_pool(name="wvt", bufs=1))
        WvT = wv_pool.tile([128, DCH, D], fr, tag="wvt")
        load3(WvT, WTv_d, D)

        # ---------- phase 2: per-tile v projection + attention ----------
        acnt = [0] * 5
        for k in range(NTILE):
            sk = SKS[k]
            vt = vpool.tile([128, D], F32, tag="v")
            for si, soff in enumerate((0, 512)):
                ps = proj_ps.tile([128, 512], F32, tag="proj")

                def vgroup(ps=ps, sk=sk, soff=soff):
                    mm0 = None
                    for dc in range(DCH):
                        m = nc.tensor.matmul(ps, xTv[:, dc, sk:sk + 128],
                                             WvT[:, dc, soff:soff + 512],
                                             start=(dc == 0), stop=(dc == DCH - 1))
                        if dc == 0:
                            mm0 = m
                    return mm0

                extra = [WvT[:, 0, :]] if (k == 0 and si == 0) else []
                guarded_matmul(vgroup, *(([xTv[:, 0, :]] if k == 0 else []) + extra))
                pe_flush(lambda vt=vt, ps=ps, soff=soff: nc.vector.tensor_tensor(
                    out=vt[:, soff:soff + 512], in0=ps,
                    in1=bvb[:, soff:soff + 512],
                    op=mybir.AluOpType.add))

            stg = stpool.tile([TS, H * L], F32, tag="stg")
            sring = k % 16
            s_all = spool.tile([TS, H * 128], F32, tag="ssb")
            blks = []
            for h in range(H):
                hs = SRC_K[h]
                sps = att_ps.tile([TS, 128], F32, tag="att")
                guarded_matmul(
                    lambda sps=sps, h=h, hs=hs, sk=sk: nc.tensor.matmul(
                        sps, qT[:, h, sk:sk + TS], kT[:, hs, sk:sk + 128],
                        start=True, stop=True))
                pe_flush(lambda h=h, sps=sps: nc.scalar.copy(
                    s_all[:, h * 128:(h + 1) * 128], sps))
            nc.scalar.dma_start(
                out=bass.AP(tensor=SS_d, offset=sring * TS * 1024,
                            ap=[[1024, TS], [1, 1024]]),
                in_=s_all)
            band_all = bpool.tile([TS, H, 9], F32, tag="band")
            # class-0 heads (0-3) share coff=4: one batched diagonal gather
            nc.scalar.dma_start(
                out=bass.AP(tensor=band_all.tensor, offset=band_all.offset,
                            ap=[[H * 9, TS], [9, 4], [1, 9]]),
                in_=bass.AP(tensor=SS_d, offset=sring * TS * 1024 + 4,
                            ap=[[1025, TS], [128, 4], [1, 9]]))
            for h in range(4, H):
                coff = C0[h] + 8
                nc.scalar.dma_start(
                    out=bass.AP(tensor=band_all.tensor,
                                offset=band_all.offset + h * 9,
                                ap=[[H * 9, TS], [1, 9]]),
                    in_=bass.AP(tensor=SS_d,
                                offset=sring * TS * 1024 + h * 128 + coff,
                                ap=[[1025, TS], [1, 9]]))
            band5_view = bass.AP(tensor=band_all.tensor, offset=band_all.offset,
                                 ap=[[H * 9, TS], [9, H], [2, L]])
            work_all = bpool.tile([TS, H * L], F32, tag="work5")
            nc.vector.tensor_tensor(out=work_all, in0=band5_view,
                                    in1=qer_all[:, k, :],
                                    op=mybir.AluOpType.add)
            e_all = bpool.tile([TS, H * L], F32, tag="ex")
            nc.scalar.activation(out=e_all, in_=work_all,
                                 func=mybir.ActivationFunctionType.Exp)
            ssum = bpool.tile([TS, H], F32, tag="rr")
            nc.vector.tensor_reduce(
                out=ssum,
                in_=bass.AP(tensor=e_all.tensor, offset=e_all.offset,
                            ap=[[H * L, TS], [L, H], [1, L]]),
                axis=mybir.AxisListType.X, op=mybir.AluOpType.add)
            rr = bpool.tile([TS, H], F32, tag="rr2")
            nc.vector.reciprocal(out=rr, in_=ssum)
            rr_bcast = bass.AP(tensor=rr.tensor, offset=rr.offset,
                               ap=[[H, TS], [1, H], [0, L]])
            nc.vector.tensor_tensor(out=stg, in0=e_all, in1=rr_bcast,
                                    op=mybir.AluOpType.mult)
            osb_all = opool.tile([TS, H * 128], F32, tag="osb")
            for h in range(H):
                cls = CLASS_OF[h]
                coff = C0[h] + 8
                blk = ABASE[cls] + (acnt[cls] % ABUFS[cls])
                acnt[cls] += 1
                blk_off = blk * 128 * TS
                nc.scalar.dma_start(
                    out=bass.AP(tensor=AZ_d, offset=blk_off + coff * TS,
                                ap=[[TS + 1, TS], [2 * TS, L], [1, 1]]),
                    in_=bass.AP(tensor=stg.tensor,
                                offset=stg.offset + h * L,
                                ap=[[H * L, TS], [1, L], [1, 1]]))
                asb = apool.tile([128, TS], F32, tag="asb")
                nc.scalar.dma_start(out=asb, in_=bass.AP(
                    tensor=AZ_d, offset=blk_off, ap=[[TS, 128], [1, TS]]))
                ops = att_ps.tile([TS, 128], F32, tag="att")
                guarded_matmul(
                    lambda ops=ops, h=h, asb=asb, vt=vt: nc.tensor.matmul(
                        ops, asb, vt[:, h * 128:(h + 1) * 128],
                        start=True, stop=True))
                pe_flush(lambda h=h, osb_all=osb_all, ops=ops: nc.scalar.copy(
                    osb_all[:, h * 128:(h + 1) * 128], ops))
                if debug and k == 3 and h == 0:
                    nc.sync.dma_start(out=DSSB_d.ap(), in_=s_all[:, 0:128])
                    nc.sync.dma_start(out=DBAND_d.ap(), in_=band_all[:, 0, :])
                    nc.sync.dma_start(out=DEX_d.ap()[:, 0:L], in_=e_all[:, 0:L])
                    nc.sync.dma_start(out=DEX_d.ap()[:, L:L + 1], in_=ssum[:, 0:1])
                    nc.sync.dma_start(out=DASB_d.ap(), in_=asb)
                    nc.sync.dma_start(out=DVT_d.ap(), in_=vt)
            nc.scalar.dma_start(out=OUT_d.ap()[sk:sk + TS, :], in_=osb_all)
            nc.scalar.dma_start(
                out=bass.AP(tensor=ATTN_d, offset=sk * L,
                            ap=[[L, TS], [TC * L, H], [1, L]]),
                in_=bass.AP(tensor=stg.tensor, offset=stg.offset,
                            ap=[[H * L, TS], [L, H], [1, L]]))
    # fp32r matmuls are self-loading (no InstLdweights); bacc's
    # move_matmul_waits_to_ldweights would drop their excess waits entirely.
    # Skip it and let generate_event_semaphores split multi-waits instead.
    nc.move_matmul_waits_to_ldweights = lambda: None
    nc.compile()
    return nc


def _round_f32r(a):
    """fp32r operands must be pre-rounded; TRN2 fp32r keeps a tf32-like
    reduced mantissa.  Round-to-nearest-even at bit 13 by default; the
    exact width is patched by _set_f32r_bits() once measured."""
    b = np.ascontiguousarray(a, dtype=np.float32).view(np.uint32).copy()
    nbits = _CACHE.get("f32r_drop_bits", 13)
    if nbits == 0:
        return b.view(np.float32)
    half = np.uint32(1 << (nbits - 1))
    lsb = (b >> np.uint32(nbits)) & np.uint32(1)
    b = b + (half - np.uint32(1)) + lsb
    b &= np.uint32(0xFFFFFFFF) << np.uint32(nbits)
    return b.view(np.float32)


def _host_prep(query, key, value, Wq, bq, Wk, bk, Wv, bv, Er):
    tau = TAU
    Wqs = Wq / tau
    bqs_host = (bq / tau).astype(np.float32)
    G = np.zeros((D, H * L), np.float32)
    qer_bias = np.zeros((H, L), np.float32)
    for h in range(H):
        G[:, h * L:(h + 1) * L] = Wqs[h * HD:(h + 1) * HD, :].T @ Er[h]
        qer_bias[h] = bqs_host[h * HD:(h + 1) * HD] @ Er[h]

    def t8(a, ncols):   # [ncols, D] -> [DCH, 128, ncols]
        return np.ascontiguousarray(a.T).reshape(DCH, 128, ncols)

    rnd = lambda x: np.ascontiguousarray(x, np.float32)
    WTq_h = rnd(t8(Wqs, D))
    WTk_h = rnd(t8(Wk, D))
    WTv_h = rnd(t8(Wv, D))
    G_h = rnd(G.reshape(DCH, 128, H * L))
    az = np.zeros((NABLK, 128, TS), np.float32)
    ss_scratch = np.zeros((16, TS, 1024), np.float32)

    in_maps = []
    for core in range(NCORES):
        b = core // 4
        t0c = TC * (core % 4)
        xq = query[b, t0c:t0c + TC, :]
        xk = np.zeros((TKV, D), np.float32)
        xv = np.zeros((TKV, D), np.float32)
        lo, hi = t0c - HALO, t0c + TC + HALO
        slo, shi = max(lo, 0), min(hi, T)
        xk[slo - lo:shi - lo] = key[b, slo:shi]
        xv[slo - lo:shi - lo] = value[b, slo:shi]

        mask = np.zeros((NTILE, TS, H * L), np.float32)
        for k in range(NTILE):
            g = t0c + SKS[k] + np.arange(TS)[:, None]
            for h in range(H):
                idx = (g + (SHIFTS[h] + np.arange(L)[None, :]) * DIL) % TP
                valid = (idx >= PADR) & (idx < T + PADR)
                mask[k, :, h * L:(h + 1) * L] = np.where(
                    valid, qer_bias[h][None, :], np.float32(-1e30))

        in_maps.append({
            "xTq": rnd(t8(xq, TC)),
            "xTk": rnd(t8(xk, TKV)),
            "xTv": rnd(t8(xv, TKV)),
            "WTq": WTq_h, "WTk": WTk_h, "WTv": WTv_h,
            "G": G_h,
            "maskadd": mask,
            "bqs": np.ascontiguousarray(bqs_host.reshape(H, 128).T),
            "bk": np.ascontiguousarray(np.asarray(bk, np.float32)[:KH * 128].reshape(KH, 128).T),
            "bv": np.ascontiguousarray(np.broadcast_to(np.asarray(bv, np.float32), (128, D))),
            "azero": az,
            "sscratch": ss_scratch,
        })
    return in_maps


def kernel(query, key, value, Wq, bq, Wk, bk, Wv, bv, Er, layer, **run_kwargs):
    assert int(layer) == LAYER
    in_maps = _host_prep(
        np.asarray(query, np.float32), np.asarray(key, np.float32),
        np.asarray(value, np.float32),
        np.asarray(Wq, np.float32), np.asarray(bq, np.float32),
        np.asarray(Wk, np.float32), np.asarray(bk, np.float32),
        np.asarray(Wv, np.float32), np.asarray(bv, np.float32),
        np.asarray(Er, np.float32))
    if "nc" not in _CACHE:
        _CACHE["nc"] = _build_nc()
    nc = _CACHE["nc"]
    res = bass_utils.run_bass_kernel_spmd(nc, in_maps, core_ids=list(range(NCORES)),
                                          **run_kwargs)
    out = np.zeros((B, T, D), np.float32)
    attn = np.zeros((B, H, T, 1, L), np.float32)
    for core in range(NCORES):
        b = core // 4
        t0c = TC * (core % 4)
        out[b, t0c:t0c + TC] = res.results[core]["out"]
        attn[b, :, t0c:t0c + TC, 0, :] = res.results[core]["attn"]
    kernel.last_result = res
    return out, attn


# revision 26
# speedup vs baseline: 3.1292x; 1.0810x over previous
"""Trainium2 Bass kernel: dilated multi-head self-attention with relative
positional embedding (window 5, dilation 2, per-head shifts, head 7 reuses
head 6 keys).

Sharding: 8 cores = 2 batches x 4 token-quarters (1024 tokens each, all 8
heads per core).  The host pre-transposes x and W (pure layout prep) so the
device spends its time on: QKV projections (fp32r matmuls at full PE rate),
banded QK scores per 112-token tile, band extraction via diagonal-stride DMA
reads, a 5-wide band softmax, a banded attention matrix rebuilt via DMA
scatter through pre-zeroed DRAM blocks, and AV matmuls.
"""
import sys

sys.path.insert(0, "/opt/trn_rl_repo")

import numpy as np
import concourse.bass as bass
from concourse import bacc
import concourse.mybir as mybir
import concourse.tile as tile
from concourse import bass_utils
from concourse.tile_rust import add_dep_helper
from contextlib import ExitStack

# ---------------- problem constants (hardcoded) ----------------
B, T, D = 2, 4096, 1024
H, HD, L = 8, 128, 5
LAYER = 1
DIL = 2 ** LAYER          # 2
PADR = DIL * (L // 2)     # 4  (reference's pad)
TP = T + 2 * PADR         # 4104
TAU = float(np.sqrt(HD))
SHIFTS = [0, 0, 0, 0, -2, -1, 1, 2]
SRC_K = [0, 1, 2, 3, 4, 5, 6, 6]
C0 = [s * DIL - PADR for s in SHIFTS]   # [-4,-4,-4,-4,-8,-6,-2,0]
NCORES = 8
TC = T // 4               # 1024 tokens per core
TS = 112                  # q-tile size
NTILE = 10
SKS = [min(k * TS, TC - TS) for k in range(NTILE)]  # 0..896, 912 (overlap)
HALO = 8
TKV = TC + 2 * HALO       # 1040
KH = 7                    # distinct k heads (head 7 reuses 6)
CLASS_OF = [0, 0, 0, 0, 1, 2, 3, 4]
ABUFS = [8, 4, 4, 4, 4]   # DRAM A-block rotation depth per class
ABASE = [0, 8, 12, 16, 20]
NABLK = 24
DCH = 8                   # D chunks of 128

F32 = mybir.dt.float32
F32R = mybir.dt.float32r

USE_F32R = True

_CACHE = {}


def _build_nc(debug=False):
    nc = bacc.Bacc(target_bir_lowering=False)
    fr = F32R if USE_F32R else F32

    # ---------- DRAM I/O ----------
    xTq_d = nc.dram_tensor("xTq", [DCH, 128, TC], fr, kind="ExternalInput")
    xTk_d = nc.dram_tensor("xTk", [DCH, 128, TKV], fr, kind="ExternalInput")
    xTv_d = nc.dram_tensor("xTv", [DCH, 128, TKV], fr, kind="ExternalInput")
    WTq_d = nc.dram_tensor("WTq", [DCH, 128, D], fr, kind="ExternalInput")
    WTk_d = nc.dram_tensor("WTk", [DCH, 128, D], fr, kind="ExternalInput")
    WTv_d = nc.dram_tensor("WTv", [DCH, 128, D], fr, kind="ExternalInput")
    G_d = nc.dram_tensor("G", [DCH, 128, H * L], fr, kind="ExternalInput")
    MASK_d = nc.dram_tensor("maskadd", [NTILE, TS, H * L], F32, kind="ExternalInput")
    BQ_d = nc.dram_tensor("bqs", [128, H], F32, kind="ExternalInput")
    BK_d = nc.dram_tensor("bk", [128, KH], F32, kind="ExternalInput")
    BV_d = nc.dram_tensor("bv", [128, D], F32, kind="ExternalInput")
    AZ_d = nc.dram_tensor("azero", [NABLK, 128, TS], F32, kind="ExternalInput")
    SS_d = nc.dram_tensor("sscratch", [16, TS, 1024], F32, kind="ExternalInput")
    OUT_d = nc.dram_tensor("out", [TC, D], F32, kind="ExternalOutput")
    if debug:
        DQT_d = nc.dram_tensor("d_qT", [128, H, TC], F32, kind="ExternalOutput")
        DKT_d = nc.dram_tensor("d_kT", [128, KH, TKV], F32, kind="ExternalOutput")
        DQER_d = nc.dram_tensor("d_qer", [TS, NTILE, H * L], F32, kind="ExternalOutput")
        DVT_d = nc.dram_tensor("d_vt", [128, D], F32, kind="ExternalOutput")
        DSSB_d = nc.dram_tensor("d_ssb", [TS, 128], F32, kind="ExternalOutput")
        DBAND_d = nc.dram_tensor("d_band", [TS, 9], F32, kind="ExternalOutput")
        DEX_d = nc.dram_tensor("d_ex", [TS, L + 1], F32, kind="ExternalOutput")
        DASB_d = nc.dram_tensor("d_asb", [128, TS], F32, kind="ExternalOutput")
    ATTN_d = nc.dram_tensor("attn", [H, TC, L], F32, kind="ExternalOutput")

    with tile.TileContext(nc) as tc, ExitStack() as ctx:
        # ---------- persistent SBUF ----------
        xTv = ctx.enter_context(nc.sbuf_tensor("xTv_sb", [128, DCH, TKV], fr)).ap()
        qT = ctx.enter_context(nc.sbuf_tensor("qT_sb", [128, H, TC], F32)).ap()
        kT = ctx.enter_context(nc.sbuf_tensor("kT_sb", [128, KH, TKV], F32)).ap()
        G_sb = ctx.enter_context(nc.sbuf_tensor("G_sb", [128, DCH, H * L], fr)).ap()
        bqs = ctx.enter_context(nc.sbuf_tensor("bqs_sb", [128, H], F32)).ap()
        bks = ctx.enter_context(nc.sbuf_tensor("bks_sb", [128, KH], F32)).ap()
        bvb = ctx.enter_context(nc.sbuf_tensor("bvb_sb", [128, D], F32)).ap()
        qer_all = ctx.enter_context(nc.sbuf_tensor("qer_all_sb", [TS, NTILE, H * L], F32)).ap()
        scratch = ctx.enter_context(nc.psum_tensor("scratch_ps", [1, 2], F32)).ap()

        wpool = ctx.enter_context(tc.tile_pool(name="wt", bufs=2))
        bpool = ctx.enter_context(tc.tile_pool(name="small", bufs=6))

        proj_ps = ctx.enter_context(tc.tile_pool(name="proj_ps", bufs=2, space="PSUM"))
        qer_ps = ctx.enter_context(tc.tile_pool(name="qer_ps", bufs=1, space="PSUM"))
        att_ps = ctx.enter_context(tc.tile_pool(name="att_ps", bufs=4, space="PSUM"))

        flushsrc = ctx.enter_context(nc.sbuf_tensor("flushsrc", [1, 2], F32)).ap()
        nc.vector.memset(flushsrc, 0.0)

        def guarded_matmul(mm_fn, *fresh):
            # bacc's generate_event_semaphores legalizes multi-wait instrs
            return mm_fn()

        def pe_flush(reader_fn):
            f = nc.tensor.matmul(scratch[0:1, 0:1], flushsrc[0:1, 0:1],
                                 flushsrc[0:1, 0:1], start=True, stop=True,
                                 skip_group_check=True)
            r = reader_fn()
            add_dep_helper(r.ins, f.ins, sync=True, reason="pe-drain-flush")
            return r

        # ---------- load persistent inputs ----------
        def load3(dst, src_d, ncols):
            # dst sbuf [128, DCH, ncols] <- src dram [DCH, 128, ncols], per chunk
            for dc in range(DCH):
                nc.sync.dma_start(
                    out=dst[:, dc, :],
                    in_=bass.AP(tensor=src_d, offset=dc * 128 * ncols,
                                ap=[[ncols, 128], [1, ncols]]))

        load3(xTv, xTv_d, TKV)
        load3(G_sb, G_d, H * L)
        nc.sync.dma_start(out=bqs, in_=BQ_d.ap())
        nc.sync.dma_start(out=bks, in_=BK_d.ap())
        nc.sync.dma_start(out=bvb, in_=BV_d.ap())

        # ---------- phase 1: q/k projections + qer ----------
        def proj_heads(dst3, w_dram, h0, nh, bias_sb, xsrc, spans):
            wt = wpool.tile([128, DCH, 4 * 128], fr, tag="wt")
            for dc in range(DCH):
                nc.sync.dma_start(
                    out=wt[:, dc, 0:nh * 128],
                    in_=bass.AP(tensor=w_dram, offset=dc * 128 * D + h0 * 128,
                                ap=[[D, 128], [1, nh * 128]]))
            for hh in range(nh):
                h = h0 + hh
                for si, (soff, slen) in enumerate(spans):
                    ps = proj_ps.tile([128, 512], F32, tag="proj")

                    def group(ps=ps, soff=soff, slen=slen, hh=hh):
                        mm0 = None
                        for dc in range(DCH):
                            m = nc.tensor.matmul(
                                ps[:, 0:slen],
                                wt[:, dc, hh * 128:(hh + 1) * 128],
                                xsrc[:, dc, soff:soff + slen],
                                start=(dc == 0), stop=(dc == DCH - 1))
                            if dc == 0:
                                mm0 = m
                        return mm0

                    guarded_matmul(group)
                    pe_flush(lambda ps=ps, soff=soff, slen=slen, h=h:
                             nc.vector.tensor_scalar(
                                 out=dst3[:, h, soff:soff + slen],
                                 in0=ps[:, 0:slen],
                                 scalar1=bias_sb[:, h:h + 1], scalar2=None,
                                 op0=mybir.AluOpType.add))

        qspans = [(0, 512), (512, 512)]
        kspans = [(0, 512), (512, 512), (1024, 16)]
        with tc.tile_pool(name="xqk", bufs=1) as xqk_pool:
            xTq = xqk_pool.tile([128, DCH, TC], fr, tag="xq")
            xTk = xqk_pool.tile([128, DCH, TKV], fr, tag="xk")
            load3(xTq, xTq_d, TC)
            load3(xTk, xTk_d, TKV)
            proj_heads(qT, WTq_d, 0, 4, bqs, xTq, qspans)
            proj_heads(qT, WTq_d, 4, 4, bqs, xTq, qspans)
            proj_heads(kT, WTk_d, 0, 4, bks, xTk, kspans)
            proj_heads(kT, WTk_d, 4, 3, bks, xTk, kspans)

            # qer for all tiles (Er folded via host-precomputed G)
            for k in range(NTILE):
                sk = SKS[k]
                qe = qer_ps.tile([TS, H * L], F32, tag="qer")

                def qgroup(qe=qe, sk=sk):
                    mm0 = None
                    for dc in range(DCH):
                        m = nc.tensor.matmul(qe, xTq[:, dc, sk:sk + TS],
                                             G_sb[:, dc, :],
                                             start=(dc == 0), stop=(dc == DCH - 1))
                        if dc == 0:
                            mm0 = m
                    return mm0

                guarded_matmul(qgroup, *([G_sb[:, 0, :]] if k == 0 else []))
                msk = bpool.tile([TS, H * L], F32, tag="msk")
                nc.sync.dma_start(out=msk, in_=MASK_d.ap()[k])
                pe_flush(lambda k=k, qe=qe, msk=msk: nc.vector.tensor_tensor(
                    out=qer_all[:, k, :], in0=qe, in1=msk,
                    op=mybir.AluOpType.add))

        if debug:
            for h in range(H):
                nc.sync.dma_start(out=DQT_d.ap()[:, h, :], in_=qT[:, h, :])
            for h in range(KH):
                nc.sync.dma_start(out=DKT_d.ap()[:, h, :], in_=kT[:, h, :])
            for k in range(NTILE):
                nc.sync.dma_start(out=DQER_d.ap()[:, k, :], in_=qer_all[:, k, :])

        # phase-2 pools allocate into the space released by xqk_pool
        vpool = ctx.enter_context(tc.tile_pool(name="vsb", bufs=2))
        spool = ctx.enter_context(tc.tile_pool(name="ssb", bufs=2))
        apool = ctx.enter_context(tc.tile_pool(name="asb", bufs=3))
        stpool = ctx.enter_context(tc.tile_pool(name="stage", bufs=4))
        opool = ctx.enter_context(tc.tile_pool(name="osb", bufs=3))
        wv_pool = ctx.enter_context(tc.tile_pool(name="wvt", bufs=1))
        WvT = wv_pool.tile([128, DCH, D], fr, tag="wvt")
        load3(WvT, WTv_d, D)

        # ---------- phase 2: per-tile v projection + attention ----------
        acnt = [0] * 5
        for k in range(NTILE):
            sk = SKS[k]
            vt = vpool.tile([128, D], F32, tag="v")
            for si, soff in enumerate((0, 512)):
                ps = proj_ps.tile([128, 512], F32, tag="proj")

                def vgroup(ps=ps, sk=sk, soff=soff):
                    mm0 = None
                    for dc in range(DCH):
                        m = nc.tensor.matmul(ps, xTv[:, dc, sk:sk + 128],
                                             WvT[:, dc, soff:soff + 512],
                                             start=(dc == 0), stop=(dc == DCH - 1))
                        if dc == 0:
                            mm0 = m
                    return mm0

                extra = [WvT[:, 0, :]] if (k == 0 and si == 0) else []
                guarded_matmul(vgroup, *(([xTv[:, 0, :]] if k == 0 else []) + extra))
                pe_flush(lambda vt=vt, ps=ps, soff=soff: nc.vector.tensor_tensor(
                    out=vt[:, soff:soff + 512], in0=ps,
                    in1=bvb[:, soff:soff + 512],
                    op=mybir.AluOpType.add))

            stg = stpool.tile([TS, H * L], F32, tag="stg")
            sring = k % 16
            s_all = spool.tile([TS, H * 128], F32, tag="ssb")
            blks = []
            for h in range(H):
                hs = SRC_K[h]
                sps = att_ps.tile([TS, 128], F32, tag="att")
                guarded_matmul(
                    lambda sps=sps, h=h, hs=hs, sk=sk: nc.tensor.matmul(
                        sps, qT[:, h, sk:sk + TS], kT[:, hs, sk:sk + 128],
                        start=True, stop=True))
                pe_flush(lambda h=h, sps=sps: nc.scalar.copy(
                    s_all[:, h * 128:(h + 1) * 128], sps))
            nc.scalar.dma_start(
                out=bass.AP(tensor=SS_d, offset=sring * TS * 1024,
                            ap=[[1024, TS], [1, 1024]]),
                in_=s_all)
            band_all = bpool.tile([TS, H, 9], F32, tag="band")
            # class-0 heads (0-3) share coff=4: one batched diagonal gather
            nc.scalar.dma_start(
                out=bass.AP(tensor=band_all.tensor, offset=band_all.offset,
                            ap=[[H * 9, TS], [9, 4], [1, 9]]),
                in_=bass.AP(tensor=SS_d, offset=sring * TS * 1024 + 4,
                            ap=[[1025, TS], [128, 4], [1, 9]]))
            for h in range(4, H):
                coff = C0[h] + 8
                nc.scalar.dma_start(
                    out=bass.AP(tensor=band_all.tensor,
                                offset=band_all.offset + h * 9,
                                ap=[[H * 9, TS], [1, 9]]),
                    in_=bass.AP(tensor=SS_d,
                                offset=sring * TS * 1024 + h * 128 + coff,
                                ap=[[1025, TS], [1, 9]]))
            band5_view = bass.AP(tensor=band_all.tensor, offset=band_all.offset,
                                 ap=[[H * 9, TS], [9, H], [2, L]])
            work_all = bpool.tile([TS, H * L], F32, tag="work5")
            nc.vector.tensor_tensor(out=work_all, in0=band5_view,
                                    in1=qer_all[:, k, :],
                                    op=mybir.AluOpType.add)
            e_all = bpool.tile([TS, H * L], F32, tag="ex")
            nc.scalar.activation(out=e_all, in_=work_all,
                                 func=mybir.ActivationFunctionType.Exp)
            ssum = bpool.tile([TS, H], F32, tag="rr")
            nc.vector.tensor_reduce(
                out=ssum,
                in_=bass.AP(tensor=e_all.tensor, offset=e_all.offset,
                            ap=[[H * L, TS], [L, H], [1, L]]),
                axis=mybir.AxisListType.X, op=mybir.AluOpType.add)
            rr = bpool.tile([TS, H], F32, tag="rr2")
            nc.vector.reciprocal(out=rr, in_=ssum)
            rr_bcast = bass.AP(tensor=rr.tensor, offset=rr.offset,
                               ap=[[H, TS], [1, H], [0, L]])
            nc.vector.tensor_tensor(out=stg, in0=e_all, in1=rr_bcast,
                                    op=mybir.AluOpType.mult)
            osb_all = opool.tile([TS, H * 128], F32, tag="osb")
            # scatters: class-0 heads (0-3) use 4 contiguous blocks per tile
            blk_of = {}
            for h in range(H):
                cls = CLASS_OF[h]
                coff = C0[h] + 8
                if h < 4:
                    blk = (k % 2) * 4 + h
                else:
                    blk = ABASE[cls] + (acnt[cls] % ABUFS[cls])
                    acnt[cls] += 1
                blk_of[h] = blk
                nc.scalar.dma_start(
                    out=bass.AP(tensor=AZ_d, offset=blk * 128 * TS + coff * TS,
                                ap=[[TS + 1, TS], [2 * TS, L], [1, 1]]),
                    in_=bass.AP(tensor=stg.tensor,
                                offset=stg.offset + h * L,
                                ap=[[H * L, TS], [1, L], [1, 1]]))
            # one batched A-load for heads 0-3, singles for 4-7
            asb4 = apool.tile([128, 4, TS], F32, tag="asb4")
            nc.scalar.dma_start(
                out=bass.AP(tensor=asb4.tensor, offset=asb4.offset,
                            ap=[[4 * TS, 128], [TS, 4], [1, TS]]),
                in_=bass.AP(tensor=AZ_d, offset=blk_of[0] * 128 * TS,
                            ap=[[TS, 128], [128 * TS, 4], [1, TS]]))
            for h in range(H):
                coff = C0[h] + 8
                if h < 4:
                    asb = asb4[:, h, :]
                else:
                    asb = apool.tile([128, TS], F32, tag="asb")
                    nc.scalar.dma_start(out=asb, in_=bass.AP(
                        tensor=AZ_d, offset=blk_of[h] * 128 * TS,
                        ap=[[TS, 128], [1, TS]]))
                ops = att_ps.tile([TS, 128], F32, tag="att")
                guarded_matmul(
                    lambda ops=ops, h=h, asb=asb, vt=vt: nc.tensor.matmul(
                        ops, asb, vt[:, h * 128:(h + 1) * 128],
                        start=True, stop=True))
                pe_flush(lambda h=h, osb_all=osb_all, ops=ops: nc.scalar.copy(
                    osb_all[:, h * 128:(h + 1) * 128], ops))
                if debug and k == 3 and h == 0:
                    nc.sync.dma_start(out=DSSB_d.ap(), in_=s_all[:, 0:128])
                    nc.sync.dma_start(out=DBAND_d.ap(), in_=band_all[:, 0, :])
                    nc.sync.dma_start(out=DEX_d.ap()[:, 0:L], in_=e_all[:, 0:L])
                    nc.sync.dma_start(out=DEX_d.ap()[:, L:L + 1], in_=ssum[:, 0:1])
                    nc.sync.dma_start(out=DASB_d.ap(), in_=asb)
                    nc.sync.dma_start(out=DVT_d.ap(), in_=vt)
            nc.scalar.dma_start(out=OUT_d.ap()[sk:sk + TS, :], in_=osb_all)
            nc.scalar.dma_start(
                out=bass.AP(tensor=ATTN_d, offset=sk * L,
                            ap=[[L, TS], [TC * L, H], [1, L]]),
                in_=bass.AP(tensor=stg.tensor, offset=stg.offset,
                            ap=[[H * L, TS], [L, H], [1, L]]))
    # fp32r matmuls are self-loading (no InstLdweights); bacc's
    # move_matmul_waits_to_ldweights would drop their excess waits entirely.
    # Skip it and let generate_event_semaphores split multi-waits instead.
    nc.move_matmul_waits_to_ldweights = lambda: None
    nc.compile()
    return nc


def _round_f32r(a):
    """fp32r operands must be pre-rounded; TRN2 fp32r keeps a tf32-like
    reduced mantissa.  Round-to-nearest-even at bit 13 by default; the
    exact width is patched by _set_f32r_bits() once measured."""
    b = np.ascontiguousarray(a, dtype=np.float32).view(np.uint32).copy()
    nbits = _CACHE.get("f32r_drop_bits", 13)
    if nbits == 0:
        return b.view(np.float32)
    half = np.uint32(1 << (nbits - 1))
    lsb = (b >> np.uint32(nbits)) & np.uint32(1)
    b = b + (half - np.uint32(1)) + lsb
    b &= np.uint32(0xFFFFFFFF) << np.uint32(nbits)
    return b.view(np.float32)


def _host_prep(query, key, value, Wq, bq, Wk, bk, Wv, bv, Er):
    tau = TAU
    Wqs = Wq / tau
    bqs_host = (bq / tau).astype(np.float32)
    G = np.zeros((D, H * L), np.float32)
    qer_bias = np.zeros((H, L), np.float32)
    for h in range(H):
        G[:, h * L:(h + 1) * L] = Wqs[h * HD:(h + 1) * HD, :].T @ Er[h]
        qer_bias[h] = bqs_host[h * HD:(h + 1) * HD] @ Er[h]

    def t8(a, ncols):   # [ncols, D] -> [DCH, 128, ncols]
        return np.ascontiguousarray(a.T).reshape(DCH, 128, ncols)

    rnd = lambda x: np.ascontiguousarray(x, np.float32)
    WTq_h = rnd(t8(Wqs, D))
    WTk_h = rnd(t8(Wk, D))
    WTv_h = rnd(t8(Wv, D))
    G_h = rnd(G.reshape(DCH, 128, H * L))
    az = np.zeros((NABLK, 128, TS), np.float32)
    ss_scratch = np.zeros((16, TS, 1024), np.float32)

    in_maps = []
    for core in range(NCORES):
        b = core // 4
        t0c = TC * (core % 4)
        xq = query[b, t0c:t0c + TC, :]
        xk = np.zeros((TKV, D), np.float32)
        xv = np.zeros((TKV, D), np.float32)
        lo, hi = t0c - HALO, t0c + TC + HALO
        slo, shi = max(lo, 0), min(hi, T)
        xk[slo - lo:shi - lo] = key[b, slo:shi]
        xv[slo - lo:shi - lo] = value[b, slo:shi]

        mask = np.zeros((NTILE, TS, H * L), np.float32)
        for k in range(NTILE):
            g = t0c + SKS[k] + np.arange(TS)[:, None]
            for h in range(H):
                idx = (g + (SHIFTS[h] + np.arange(L)[None, :]) * DIL) % TP
                valid = (idx >= PADR) & (idx < T + PADR)
                mask[k, :, h * L:(h + 1) * L] = np.where(
                    valid, qer_bias[h][None, :], np.float32(-1e30))

        in_maps.append({
            "xTq": rnd(t8(xq, TC)),
            "xTk": rnd(t8(xk, TKV)),
            "xTv": rnd(t8(xv, TKV)),
            "WTq": WTq_h, "WTk": WTk_h, "WTv": WTv_h,
            "G": G_h,
            "maskadd": mask,
            "bqs": np.ascontiguousarray(bqs_host.reshape(H, 128).T),
            "bk": np.ascontiguousarray(np.asarray(bk, np.float32)[:KH * 128].reshape(KH, 128).T),
            "bv": np.ascontiguousarray(np.broadcast_to(np.asarray(bv, np.float32), (128, D))),
            "azero": az,
            "sscratch": ss_scratch,
        })
    return in_maps


def kernel(query, key, value, Wq, bq, Wk, bk, Wv, bv, Er, layer, **run_kwargs):
    assert int(layer) == LAYER
    in_maps = _host_prep(
        np.asarray(query, np.float32), np.asarray(key, np.float32),
        np.asarray(value, np.float32),
        np.asarray(Wq, np.float32), np.asarray(bq, np.float32),
        np.asarray(Wk, np.float32), np.asarray(bk, np.float32),
        np.asarray(Wv, np.float32), np.asarray(bv, np.float32),
        np.asarray(Er, np.float32))
    if "nc" not in _CACHE:
        _CACHE["nc"] = _build_nc()
    nc = _CACHE["nc"]
    res = bass_utils.run_bass_kernel_spmd(nc, in_maps, core_ids=list(range(NCORES)),
                                          **run_kwargs)
    out = np.zeros((B, T, D), np.float32)
    attn = np.zeros((B, H, T, 1, L), np.float32)
    for core in range(NCORES):
        b = core // 4
        t0c = TC * (core % 4)
        out[b, t0c:t0c + TC] = res.results[core]["out"]
        attn[b, :, t0c:t0c + TC, 0, :] = res.results[core]["attn"]
    kernel.last_result = res
    return out, attn
